# revision 43
# baseline (speedup 1.0000x reference)
"""Trainium2 Bass kernel for nn_JointNet (RNN-T joint network).

Reference computation (fp32):
    enc_proj = encoder_outputs @ W1[:D]          # [B,T,H]
    dec_proj = decoder_outputs @ W1[D:]          # [B,U,H]
    hidden   = tanh(enc_proj[:,:,None,:] + dec_proj[:,None,:,:] + b1)
    out      = hidden @ W2                       # [B,T,U,V]

Shapes (hardcoded): B=4, T=256, U=64, D=512, H=512, V=1024.

Sharding: data-parallel over (B x T/2) -> 8 shards, one per NeuronCore.
Core c handles batch b = c//2, t-range [(c%2)*128, (c%2)*128+128).
No collectives needed; host assembles the output slices.

Per-core plan (all in transposed "feature-on-partition" layout):
  1. Load enc slice [128,512], dec slice [64,512], W1 [1024,512],
     b1 [512], W2 [512,1024], spread across the SP/ACT/gpsimd DMA queues.
  2. PE-transpose enc/dec to encT/decT [d, t|u].
  3. Project: encbT[h,t] = W1_enc.T @ encT,  decbT[h,u] = W1_dec.T @ decT + b1.
  4. For each u (64 iters):
       hidT[h,t]  = tanh(encbT[h,:] + decbT[h,u])      (ScalarE, bias trick)
       psum[t,v]  = sum_h hidT[h_tile].T @ W2[h_tile]  (TensorE, fp32r)
       sbuf stage <- psum (VectorE), out[u] <- stage   (one 512KB DMA)
  Steady state is TensorE-bound: 8 back-to-back N=512 matmuls per u
  (~1.7us) with ACT/DVE/DMA fully hidden underneath.

fp32r (same bits as fp32, full PE streaming rate at free-dim>=256) is used
for all matmul operands; plain fp32 matmul runs at 1/4 rate on TRN2.
"""

import numpy as np

import concourse.bass as bass
import concourse.mybir as mybir
import concourse.tile as tile
from concourse.bass import ts
from concourse.bass_utils import run_bass_kernel_spmd
from concourse.masks import make_identity
from concourse.vector_clock import ScopedClock

B, T, U, D, H, V = 4, 256, 64, 512, 512, 1024
T_SH = 128  # t-rows per core
N_CORES = 8
F32 = mybir.dt.float32
F32R = mybir.dt.float32r
P = 128


class _SingleWaitTileContext(tile.TileContext):
    """This container's walrus build accepts only ONE sync-wait per
    instruction ("Too many sync wait commands" at codegen otherwise).
    Peel extra waits onto same-engine no-ops emitted just before the
    real instruction, and chunk the kernel-tail drain the same way."""

    def _add_instruction(self, inst):
        si = inst.sync_info
        if si is not None and si.on_wait is not None and len(si.on_wait) > 1:
            waits = list(si.on_wait)
            for w in waits[:-1]:
                nop = mybir.InstNoOp(
                    name=self.nc.get_next_instruction_name(),
                    sync_info=mybir.SyncInfo(on_wait=[w], on_update=[]),
                    bass_nofuse=True,
                    engine=inst.engine,
                )
                super()._add_instruction(nop)
            inst.sync_info = mybir.SyncInfo(
                on_wait=[waits[-1]], on_update=list(si.on_update)
            )
        super()._add_instruction(inst)

    def _drain_and_barrier(self, tick_clock, wait_clock):
        nop0 = self.nc.sync.nop(nofuse=True)
        wait_clock.add_sem_waits(
            nop0.ins, ScopedClock({None: tick_clock.global_clock})
        )
        waits = list(nop0.ins.sync_info.on_wait)
        ups = list(nop0.ins.sync_info.on_update)
        nop0.ins.sync_info = mybir.SyncInfo(on_wait=waits[:1], on_update=ups)
        for w in waits[1:]:
            nxt = self.nc.sync.nop(nofuse=True)
            nxt.ins.sync_info = mybir.SyncInfo(on_wait=[w], on_update=[])
        self.nc.sync.drain()
        self.nc.all_engine_barrier()
        assert self.sems is not None
        popped = self.nc._tile_sem_poison_stack.pop()
        assert popped is self._sem_poison
        self.nc.clear_and_free_semaphores(list(self.sems.allocated().values()))
        self.nc.all_engine_barrier()


def build_nc():
    nc = bass.Bass(trn_type="TRN2")
    enc = nc.dram_tensor("enc", [T_SH, D], F32, kind="ExternalInput")
    dec = nc.dram_tensor("dec", [U, D], F32, kind="ExternalInput")
    w1 = nc.dram_tensor("w1", [2 * D, H], F32R, kind="ExternalInput")
    b1 = nc.dram_tensor("b1", [H], F32, kind="ExternalInput")
    w2 = nc.dram_tensor("w2", [H, V], F32R, kind="ExternalInput")
    # u-major output layout: out[u] is one contiguous [T_SH, V] 512KB block
    # per main-loop iteration (single fat DMA, minimal descriptor work on the
    # SP sequencer). The host swaps (u, t) axes when assembling.
    out = nc.dram_tensor("out", [U, T_SH, V], F32, kind="ExternalOutput")

    HT = H // P  # 4 h-tiles
    DT = D // P  # 4 d-tiles

    with _SingleWaitTileContext(nc) as tc:
        with (
            tc.tile_pool(name="consts", bufs=1) as consts,
            tc.tile_pool(name="hid", bufs=16) as hidp,
            tc.tile_pool(name="ostage", bufs=6) as ostage,
            tc.tile_pool(name="pst", bufs=3, space="PSUM") as pst,
            tc.tile_pool(name="pso", bufs=5, space="PSUM") as pso,
        ):
            # ---- loads ----
            # DMA transfers serialize on the issuing engine's queue, so the
            # ~4.4MB of inputs is spread over the SP, ACT, and gpsimd queues,
            # ordered so each dependency chain starts as early as possible.
            # Identity + scrap first on gpsimd (they gate the transposes and
            # the Tanh-table preload; must not sit behind fat weight DMAs).
            ident = consts.tile([P, P], F32)
            make_identity(nc, ident[:])
            scrap = consts.tile([P, 1], F32)
            nc.gpsimd.memset(scrap[:], 0.0)
            # enc split by d-halves across SP+ACT so the first transposes can
            # start ~1us earlier (enc gates the whole PE pipeline).
            enc_sb = consts.tile([T_SH, D], F32)
            nc.sync.dma_start(enc_sb[:, : D // 2], enc[:, : D // 2])
            nc.scalar.dma_start(enc_sb[:, D // 2 :], enc[:, D // 2 :])
            dec_sb = consts.tile([U, D], F32)
            nc.sync.dma_start(dec_sb[:], dec[:])
            b1_sb = consts.tile([P, HT], F32)
            nc.sync.dma_start(b1_sb[:], b1.rearrange("(o p) -> p o", p=P))
            # W1: dec half on gpsimd (it gates the bias chain), enc on ACT.
            w1_sb = consts.tile([P, 2 * DT, H], F32R)  # [d_in, d_out, h]
            w1r = w1.rearrange("(o p) h -> p o h", p=P)
            nc.gpsimd.dma_start(w1_sb[:, DT:], w1r[:, DT:])
            nc.scalar.dma_start(w1_sb[:, :DT], w1r[:, :DT])
            # Combined projection rhs, allocated here so its pad columns can
            # be zeroed on the gpsimd queue right behind the W1 issue (only
            # cols >= 192 are read as pad; a full-tile DVE memset would queue
            # in front of the encbT copies that gate the first tanh).
            PRJ = 256
            ecdT = consts.tile([P, DT, PRJ], F32R)
            nc.gpsimd.memset(ecdT[:, :, T_SH + U :].bitcast(F32), 0.0)
            # W2 per-h chunks spread over all three DMA-capable queues.
            w2_sb = consts.tile([P, HT, V], F32R)  # [h_in, h_out, v]
            w2r = w2.rearrange("(o p) v -> p o v", p=P)
            w2_eng = [nc.sync, nc.gpsimd, nc.scalar, nc.sync]
            for h in range(HT):
                w2_eng[h].dma_start(w2_sb[:, h : h + 1], w2r[:, h : h + 1])
            # Warm the ACT Tanh table while the DMAs stream: the first real
            # tanh otherwise pays the ~1.4us table load on the critical path.
            nc.scalar.activation(
                scrap[:], scrap[:], mybir.ActivationFunctionType.Tanh
            )

            # ---- transpose enc/dec into one combined rhs [d, t(128)|u(64)|pad] ----
            # Free dim padded to 256 so the fp32r projection matmuls stream at
            # full rate (1 cycle/row needs moving dim >= 256).
            for d in range(DT):
                pt = pst.tile([P, T_SH], F32, tag="pst")
                nc.tensor.transpose(pt[:], enc_sb[:, ts(d, P)], ident[:])
                nc.vector.tensor_copy(ecdT[:, d, :T_SH], pt[:])
            for d in range(DT):
                pt = pst.tile([P, T_SH], F32, tag="pst")
                nc.tensor.transpose(pt[:, :U], dec_sb[:U, ts(d, P)], ident[:U, :U])
                nc.vector.tensor_copy(ecdT[:, d, T_SH : T_SH + U], pt[:, :U])

            # ---- projections ----
            # enc rhs streams the full padded 256 columns (cols >=128 are
            # discarded) so the fp32r matmul runs at 1 cycle/row; dec runs
            # natural N=64 (same absolute cost either way).
            encbT = consts.tile([P, HT, T_SH], F32)
            decbT = consts.tile([P, HT, U], F32)
            for h in range(HT):
                # dec first: it gates the bias columns for the first tanh.
                pd = pst.tile([P, U], F32, tag="pst")
                for d in range(DT):
                    nc.tensor.matmul(
                        pd[:], w1_sb[:, DT + d, ts(h, P)], ecdT[:, d, T_SH : T_SH + U],
                        start=(d == 0), stop=(d == DT - 1),
                    )
                nc.vector.tensor_scalar_add(
                    decbT[:, h], pd[:], b1_sb[:, h : h + 1]
                )
                pe = pst.tile([P, PRJ], F32, tag="pst")
                for d in range(DT):
                    nc.tensor.matmul(
                        pe[:], w1_sb[:, d, ts(h, P)], ecdT[:, d],
                        start=(d == 0), stop=(d == DT - 1),
                    )
                # DVE copy (not ACT) keeps the ACT table warm for Tanh.
                nc.vector.tensor_copy(encbT[:, h], pe[:, :T_SH])

            # ---- main loop over u ----
            # m-tile = all 128 t rows for one u. ACT op granularity is
            # [128, 128] (one bias column per u) -- ACT fixed overhead
            # (~300ns/op) makes smaller ops the bottleneck.
            for u in range(U):
                hids = []
                for h in range(HT):
                    ht = hidp.tile([P, T_SH], F32R, tag="hid")
                    nc.scalar.activation(
                        ht[:], encbT[:, h],
                        mybir.ActivationFunctionType.Tanh,
                        bias=decbT[:, h, u : u + 1], scale=1.0,
                    )
                    hids.append(ht)
                so = ostage.tile([P, V], F32, tag="ostage")
                for v in range(V // 512):
                    po = pso.tile([P, 512], F32, tag="pso")
                    for h in range(HT):
                        nc.tensor.matmul(
                            po[:], hids[h][:], w2_sb[:, h, ts(v, 512)],
                            start=(h == 0), stop=(h == HT - 1),
                        )
                    nc.vector.tensor_copy(so[:, ts(v, 512)], po[:])
                    if u == U - 1:
                        # tail: per-half DMAs on separate engine queues so the
                        # final transfers run concurrently.
                        eng = nc.scalar if v == 0 else nc.sync
                        eng.dma_start(out[u, :, ts(v, 512)], so[:, ts(v, 512)])
                if u != U - 1:
                    nc.sync.dma_start(out[u], so[:])
    return nc


_NC_CACHE = None


def _get_nc():
    global _NC_CACHE
    if _NC_CACHE is None:
        _NC_CACHE = build_nc()
    return _NC_CACHE


def kernel(encoder_outputs, decoder_outputs, W1, b1, W2):
    encoder_outputs = np.asarray(encoder_outputs, dtype=np.float32)
    decoder_outputs = np.asarray(decoder_outputs, dtype=np.float32)
    W1 = np.ascontiguousarray(np.asarray(W1, dtype=np.float32))
    b1 = np.ascontiguousarray(np.asarray(b1, dtype=np.float32))
    W2 = np.ascontiguousarray(np.asarray(W2, dtype=np.float32))

    nc = _get_nc()
    in_maps = []
    for c in range(N_CORES):
        b, th = divmod(c, T // T_SH)
        in_maps.append(
            {
                "enc": np.ascontiguousarray(
                    encoder_outputs[b, th * T_SH : (th + 1) * T_SH]
                ),
                "dec": np.ascontiguousarray(decoder_outputs[b]),
                "w1": W1,
                "b1": b1,
                "w2": W2,
            }
        )
    res = run_bass_kernel_spmd(nc, in_maps, core_ids=list(range(N_CORES)))
    out = np.empty((B, T, U, V), np.float32)
    for c in range(N_CORES):
        b, th = divmod(c, T // T_SH)
        # device layout is [U, T_SH, V]; swap to [T_SH, U, V]
        out[b, th * T_SH : (th + 1) * T_SH] = res.results[c]["out"].transpose(1, 0, 2)
    return out
